# revision 81
# baseline (speedup 1.0000x reference)
"""Trainium2 Bass kernel for MultiHeadAttention (B=4, S=1024, E=1024, H=16, Dh=64).

Sharding: 8 cores = (batch b in 0..3) x (head-group hg in 0..1, 8 heads each).
The reference reshapes [B,H,S,Dh] -> [B,S,E] WITHOUT transposing heads back, so
head h's attention output occupies output rows t' = h*64 + s//16 — the final
projection is row-parallel across head groups: no cross-core communication.

Fast path (causal mask, zero bq/bk — the graded configuration), ~100us/core
(from the 136us fp32r baseline):
  All matmuls fp16 (same PE rate as fp32r, no narrow-width penalty, half the
  DMA/SBUF, better mantissa) except attn@V which is bf16 (exp outputs reach
  ~1e13, beyond fp16 range).  exp(0.5*x) on ACT writes bf16 from per-512-col
  PSUM score tiles; causal fill via a 128-wide gpsimd affine_select on just
  the diagonal block.  The PE stream is hand-interleaved (score groups spread
  between K/V-projection, attn@V and out-projection units) so the
  scores->exp->attn@V chain pipelines against independent matmul work, and
  is led by dummy warmup matmuls + split-K first Q groups so the array is at
  the top p-state and busy as soon as the first weight DMA chunks land.
  Normalization (1/rowsum) reads z straight from PSUM and fuses the
  scrambled-reshape gather, writing fp16 into the out-projection stationary.
  Deep pools (4 score PSUM tiles, 24 e-tiles, 4 output staging tiles) keep
  every producer/consumer WAR chain off the critical path; the epilogue
  hides the last head's normalize under an earlier output tile and splits
  the final DMA so the post-matmul tail is ~3.5us.

Fallback (non-causal mask or nonzero bq/bk): original fp32r implementation.
"""
import numpy as np

B, S, E, H, DH = 4, 1024, 1024, 16, 64
NCORES = 8
HPC = 8          # heads per core
EC = 8           # 128-row chunks of E
TT = 8           # 128-row t-tiles of S
NJ = 2           # 512-col s-blocks

_CACHE = {}


def _computed(tt, j):
    # scores^T block (t-tile tt, s-block j) skipped iff fully masked (t > s)
    return 128 * tt <= 512 * j + 511


def _build_fast():
    import concourse.bacc as bacc
    import concourse.tile as tile
    import concourse.mybir as mybir

    f32 = mybir.dt.float32
    f16 = mybir.dt.float16
    bf16 = mybir.dt.bfloat16
    Exp = mybir.ActivationFunctionType.Exp
    Copy = mybir.ActivationFunctionType.Copy
    mult = mybir.AluOpType.mult
    is_ge = mybir.AluOpType.is_ge

    nc = bacc.Bacc("TRN2")
    xt = nc.dram_tensor("xt", [128, EC, S], f16, kind="ExternalInput")
    wq = nc.dram_tensor("wq", [128, EC, 512], f16, kind="ExternalInput")
    wk = nc.dram_tensor("wk", [128, EC, 512], f16, kind="ExternalInput")
    wv = nc.dram_tensor("wv", [128, EC, 512], f16, kind="ExternalInput")
    wo = nc.dram_tensor("wo", [128, EC, 1024], f16, kind="ExternalInput")
    out = nc.dram_tensor("out", [4, 128, 1024], f32, kind="ExternalOutput")

    with tile.TileContext(nc) as tc:
        with (
            tc.tile_pool(name="pers", bufs=1) as pp,
            tc.tile_pool(name="pj", bufs=2, space="PSUM") as pj,
            tc.tile_pool(name="sc", bufs=4, space="PSUM") as sc,
            tc.tile_pool(name="ztp", bufs=2, space="PSUM") as ztp,
            tc.tile_pool(name="expa", bufs=24) as expa,
            tc.tile_pool(name="expb", bufs=24) as expb,
            tc.tile_pool(name="small", bufs=2) as small,
            tc.tile_pool(name="outp", bufs=6) as outp,
        ):
            # NOTE: sc tiles are [128,1024] (2 PSUM banks): pj(2)+sc(2x2)+ztp(2)=8
            xt_sb = pp.tile([128, EC, S], f16)
            wq_sb = pp.tile([128, EC, 512], f16)
            wk_sb = pp.tile([128, EC, 512], f16)
            wv_sb = pp.tile([128, EC, 512], f16)
            wo_sb = pp.tile([128, EC, 1024], f16)
            qt_sb = pp.tile([128, 4, S], f16)
            kt_sb = pp.tile([128, 4, S], f16)
            vp_sb = pp.tile([128, TT, 1024], bf16)
            x2t_sb = pp.tile([128, EC, 512], f16)
            dummy = pp.tile([128, 640], f16)

            # DMA order: Q-projection deps first (half-K chunks so the split-K
            # Q groups can start early), then stream the rest.
            for k0, k1 in ((0, 2), (2, 4), (4, 8)):
                nc.sync.dma_start(out=wq_sb[:, k0:k1, :], in_=wq[:, k0:k1, :])
                nc.sync.dma_start(out=xt_sb[:, k0:k1, 0:512],
                                  in_=xt[:, k0:k1, 0:512])
            nc.sync.dma_start(out=xt_sb[:, :, 512:1024], in_=xt[:, :, 512:1024])
            nc.sync.dma_start(out=wk_sb, in_=wk.ap())
            nc.sync.dma_start(out=wv_sb, in_=wv.ap())
            nc.sync.dma_start(out=wo_sb, in_=wo.ap())

            nc.gpsimd.memset(dummy, 0.0)
            # ones everywhere; V-projection copies overwrite the value halves
            nc.gpsimd.memset(vp_sb, 1.0)
            vview = vp_sb.rearrange("p t (h two d) -> p t h two d", two=2, d=DH)

            # ---- PE p-state warmup: keep the array busy during input DMA ----
            for i in range(8):
                ps = pj.tile([128, 512], f32, tag="pj", name=f"warm_{i}")
                nc.tensor.matmul(ps, dummy[:, 0:128], dummy[:, 128:640],
                                 start=True, stop=True)

            # ---- Q^T projection: [d-tile, s] = wq.T @ X^T  (all of it) ----
            def proj_qk(wsb, dest, dt_, sh, on_act=False):
                ps = pj.tile([128, 512], f32, tag="pj")
                for ec in range(EC):
                    nc.tensor.matmul(
                        ps, wsb[:, ec, 128 * dt_:128 * dt_ + 128],
                        xt_sb[:, ec, 512 * sh:512 * sh + 512],
                        start=(ec == 0), stop=(ec == EC - 1),
                    )
                if on_act:
                    # ACT is idle before the first exps; avoids queueing the
                    # kt copy behind V-projection copies on DVE
                    nc.scalar.activation(
                        dest[:, dt_, 512 * sh:512 * sh + 512], ps, Copy)
                else:
                    nc.vector.tensor_copy(
                        dest[:, dt_, 512 * sh:512 * sh + 512], ps)

            # Q sh0 via split-K: accumulation groups stay open across 4 passes
            # so each 2-ec chunk is consumed as soon as its DMA lands.
            # The 4 tiles live in the (still idle) scores pool.
            psq = {}
            for dt_ in range(4):
                psq[dt_] = sc.tile([128, 512], f32, tag="sc", name=f"q0_{dt_}")
            for k0, k1 in ((0, 2), (2, 4), (4, 8)):
                for dt_ in range(4):
                    for ec in range(k0, k1):
                        nc.tensor.matmul(
                            psq[dt_], wq_sb[:, ec, 128 * dt_:128 * dt_ + 128],
                            xt_sb[:, ec, 0:512],
                            start=(ec == 0), stop=(ec == 7),
                        )
            for dt_ in range(4):
                nc.vector.tensor_copy(qt_sb[:, dt_, 0:512], psq[dt_])
            # qt dt0 sh1 + kt dt0: the minimal deps of pair-0 scores, so
            # the exp stream starts ~3.4us earlier
            proj_qk(wq_sb, qt_sb, 0, 1)
            proj_qk(wk_sb, kt_sb, 0, 0)
            proj_qk(wk_sb, kt_sb, 0, 1)
            for dt_ in range(1, 4):
                proj_qk(wq_sb, qt_sb, dt_, 1)

            def proj_v(tt):
                ps = pj.tile([128, 512], f32, tag="pj")
                for ec in range(EC):
                    nc.tensor.matmul(
                        ps, xt_sb[:, ec, 128 * tt:128 * tt + 128],
                        wv_sb[:, ec, :],
                        start=(ec == 0), stop=(ec == EC - 1),
                    )
                nc.vector.tensor_copy(
                    vview[:, tt, :, 0, :], ps.rearrange("p (h d) -> p h d", d=DH)
                )

            et = {}

            def scores_tt(hp, tt):
                # emit scores matmuls + exp (+ causal fill) for head pair hp,
                # t-tile tt; per-j [128,512] PSUM tiles for pipeline depth.
                pair = (2 * hp, 2 * hp + 1)
                c0 = 128 * tt
                js = [j for j in range(NJ) if _computed(tt, j)]
                s0 = 512 * js[0]
                es = {}
                for h in pair:
                    if tt < 4:
                        es[h] = expa.tile([128, 1024], bf16, tag="expa",
                                          name=f"e_{h}_{tt}")
                    else:
                        es[h] = expb.tile([128, 512], bf16, tag="expb",
                                          name=f"e_{h}_{tt}")
                for j in js:
                    for h in pair:
                        dt_ = h // 2
                        pb = 64 * (h % 2)
                        e = es[h]
                        lo = max(512 * j, c0)
                        ps = sc.tile([128, 512], f32, tag="sc",
                                     name=f"ps_{h}_{tt}_{j}")
                        nc.tensor.matmul(
                            ps[:, lo - 512 * j:512],
                            kt_sb[pb:pb + 64, dt_, c0:c0 + 128],
                            qt_sb[pb:pb + 64, dt_, lo:512 * j + 512],
                            start=True, stop=True,
                        )
                        # e columns are true s minus s0
                        nc.scalar.activation(
                            e[:, lo - s0:512 * j + 512 - s0],
                            ps[:, lo - 512 * j:512], Exp, scale=0.5,
                        )
                        if lo == c0:
                            # zero the below-diagonal triangle (only this
                            # 128-col diagonal block is read with cols < t)
                            nc.gpsimd.affine_select(
                                out=e[:, c0 - s0:c0 + 128 - s0],
                                in_=e[:, c0 - s0:c0 + 128 - s0],
                                pattern=[[1, 128]], compare_op=is_ge,
                                fill=0.0, base=0, channel_multiplier=-1,
                            )
                        et[(h, tt, j)] = e[:, 512 * j - s0:512 * j - s0 + 512]

            recs = {}

            def z_unit(h, j):
                # attn @ V for head h, s-block j (bf16), then 1/sum + scrambled
                # normalize directly from PSUM into x2t (out-proj stationary)
                zt = ztp.tile([128, 512], f32, tag="zt", name=f"zt_{h}_{j}")
                ks = [tt for tt in range(TT) if (h, tt, j) in et]
                for i, tt in enumerate(ks):
                    lo = max(0, 128 * tt - 512 * j)
                    nc.tensor.matmul(
                        zt[:, lo:], vp_sb[:, tt, 128 * h:128 * h + 128],
                        et[(h, tt, j)][:, lo:],
                        start=(i == 0), stop=(i == len(ks) - 1),
                    )
                if h not in recs:
                    recs[h] = small.tile([64, S], f32, tag="rec", name=f"rec_{h}")
                rec = recs[h]
                nc.vector.reciprocal(rec[:, 512 * j:512 * j + 512], zt[64:128, :])
                zv = zt[0:64].rearrange("p (m c par) -> par p c m", m=32, c=8, par=2)
                rv = rec[:, 512 * j:512 * j + 512].rearrange(
                    "p (m c par) -> par p c m", m=32, c=8, par=2)
                for P in range(2):
                    nc.vector.tensor_tensor(
                        x2t_sb[64 * P:64 * P + 64, :,
                               64 * h + 32 * j:64 * h + 32 * j + 32],
                        zv[P], rv[P], op=mult,
                    )

            def out_unit(tp, eh, split=1):
                ps = pj.tile([128, 512], f32, tag="pj")
                for c in range(EC):
                    nc.tensor.matmul(
                        ps, x2t_sb[:, c, 128 * tp:128 * tp + 128],
                        wo_sb[:, c, 512 * eh:512 * eh + 512],
                        start=(c == 0), stop=(c == EC - 1),
                    )
                osb = outp.tile([128, 512], f32, tag="osb")
                # split>1 pipelines copy+DMA chunks to shorten the final tail
                w = 512 // split
                for i in range(split):
                    nc.vector.tensor_copy(osb[:, i * w:(i + 1) * w],
                                          ps[:, i * w:(i + 1) * w])
                    nc.sync.dma_start(
                        out=out[tp][:, 512 * eh + i * w:512 * eh + (i + 1) * w],
                        in_=osb[:, i * w:(i + 1) * w])

            # ---- interleaved PE stream ----
            # Each phase emits pair-hp's 8 score groups ("s", tt) spaced among
            # fill work; fills lead each phase so the previous pair's exps can
            # drain (scores PSUM bufs WAR against them).
            phases = [
                # pair-0 scores x {K-proj dt1..3, V-proj tt0,1}
                (0, [("k", 1, 0), "s", ("k", 1, 1), "s", ("k", 2, 0), "s",
                     ("k", 2, 1), "s", ("k", 3, 0), "s", ("k", 3, 1), "s",
                     ("v", 0), "s", ("v", 1), "s"]),
                # pair-1 scores x {V-proj tt2..7, z h0, z h1 j0}
                (1, [("v", 2), "s", ("v", 3), "s", ("v", 4), "s", ("v", 5),
                     "s", ("v", 6), "s", ("v", 7), "s", ("z", 0, 0), "s",
                     ("z", 0, 1), "s", ("z", 1, 0)]),
                # pair-2 scores x {z h1 j1..h3, out0}
                (2, [("z", 1, 1), "s", ("o", 0, 0), "s",
                     ("o", 0, 1), "s", ("z", 2, 0), "s", ("z", 2, 1), "s",
                     ("z", 3, 0), "s", "s", ("z", 3, 1), "s"]),
                # pair-3 scores x {z h4..h5, z h6 j0, out1}
                (3, [("o", 1, 0), ("o", 1, 1), "s", ("z", 4, 0), "s",
                     ("z", 4, 1), "s", ("z", 5, 0), "s", ("z", 5, 1), "s",
                     ("z", 6, 0), "s", "s", "s"]),
            ]

            def emit(u):
                if u[0] == "k":
                    proj_qk(wk_sb, kt_sb, u[1], u[2], on_act=True)
                elif u[0] == "v":
                    proj_v(u[1])
                elif u[0] == "z":
                    z_unit(u[1], u[2])
                else:
                    out_unit(u[1], u[2])

            for hp, seq in phases:
                tt = 0
                for u in seq:
                    if u == "s":
                        scores_tt(hp, tt)
                        tt += 1
                    else:
                        emit(u)
                assert tt == TT

            # tail: j0 z-units first (their exps drained long ago), j1 units
            # as the last pair-3 exps land; out2 hides h7's normalize.
            for u in (("z", 7, 0), ("z", 6, 1), ("z", 7, 1),
                      ("o", 2, 0), ("o", 2, 1), ("o", 3, 0)):
                emit(u)
            # final half-tile: two independent 256-col groups on the idle z
            # pool so the first chunk's DMA chain starts before the last mms
            for i in range(2):
                ps = ztp.tile([128, 256], f32, tag="zt")
                for c in range(EC):
                    nc.tensor.matmul(
                        ps, x2t_sb[:, c, 384:512],
                        wo_sb[:, c, 512 + 256 * i:768 + 256 * i],
                        start=(c == 0), stop=(c == EC - 1),
                    )
                osb = outp.tile([128, 256], f32, tag="osb")
                nc.vector.tensor_copy(osb, ps)
                nc.sync.dma_start(out=out[3][:, 512 + 256 * i:768 + 256 * i],
                                  in_=osb)
    nc.compile()
    return nc


def _build_legacy(variant):
    import concourse.bacc as bacc
    import concourse.tile as tile
    import concourse.mybir as mybir

    f32 = mybir.dt.float32
    f32r = mybir.dt.float32r
    Exp = mybir.ActivationFunctionType.Exp
    mult = mybir.AluOpType.mult
    is_ge = mybir.AluOpType.is_ge

    causal = variant == "causal"

    def computed(tt, j):
        if not causal:
            return True
        return 128 * tt <= 512 * j + 511

    nc = bacc.Bacc("TRN2")
    xt = nc.dram_tensor("xt", [128, EC, S], f32r, kind="ExternalInput")
    wq = nc.dram_tensor("wq", [128, EC, 512], f32r, kind="ExternalInput")
    wk = nc.dram_tensor("wk", [128, EC, 512], f32r, kind="ExternalInput")
    wv = nc.dram_tensor("wv", [128, EC, 512], f32r, kind="ExternalInput")
    wo = nc.dram_tensor("wo", [128, EC, 1024], f32r, kind="ExternalInput")
    bq = nc.dram_tensor("bq", [128, 4], f32, kind="ExternalInput")
    bk = nc.dram_tensor("bk", [128, 4], f32, kind="ExternalInput")
    if not causal:
        mkt = nc.dram_tensor("mkt", [128, TT, S], f32, kind="ExternalInput")
    out = nc.dram_tensor("out", [4, 128, 1024], f32, kind="ExternalOutput")

    with tile.TileContext(nc) as tc:
        with (
            tc.tile_pool(name="persist", bufs=1) as pp,
            tc.tile_pool(name="mm", bufs=3, space="PSUM") as mm,
            tc.tile_pool(name="ztp", bufs=2, space="PSUM") as ztp,
        ):
            p1 = tc.alloc_tile_pool(name="p1", bufs=1)
            xt_sb = p1.tile([128, EC, S], f32r)
            wq_sb = p1.tile([128, EC, 512], f32r)
            wk_sb = p1.tile([128, EC, 512], f32r)
            wv_sb = p1.tile([128, EC, 512], f32r)
            for k in range(0, EC, 2):
                nc.sync.dma_start(out=xt_sb[:, k:k + 2, :], in_=xt[:, k:k + 2, :])
                nc.sync.dma_start(out=wq_sb[:, k:k + 2, :], in_=wq[:, k:k + 2, :])
                nc.sync.dma_start(out=wk_sb[:, k:k + 2, :], in_=wk[:, k:k + 2, :])
                nc.sync.dma_start(out=wv_sb[:, k:k + 2, :], in_=wv[:, k:k + 2, :])
            qt_sb = pp.tile([128, 4, S], f32r)
            kt_sb = pp.tile([128, 4, S], f32r)
            vp_sb = pp.tile([128, TT, 1024], f32r)
            x2t_sb = pp.tile([128, EC, 512], f32r)
            bq_sb = pp.tile([128, 4], f32)
            bk_sb = pp.tile([128, 4], f32)
            if not causal:
                mkt_sb = pp.tile([128, TT, S], f32)
                nc.sync.dma_start(out=mkt_sb, in_=mkt.ap())
            nc.sync.dma_start(out=bq_sb, in_=bq.ap())
            nc.sync.dma_start(out=bk_sb, in_=bk.ap())

            vview = vp_sb.rearrange("p t (h two d) -> p t h two d", two=2, d=DH)
            ones_sb = pp.tile([128, 512], f32)
            nc.vector.memset(ones_sb, 1.0)
            ones_v = ones_sb.rearrange("p (h d) -> p h d", d=DH)
            for tt in range(TT):
                nc.vector.tensor_copy(vview[:, tt, :, 1, :], ones_v)

            for wsb, dest, bias in ((wq_sb, qt_sb, bq_sb), (wk_sb, kt_sb, bk_sb)):
                for dt_ in range(4):
                    for sh in range(2):
                        ps = mm.tile([128, 512], f32, tag="mm")
                        for ec in range(EC):
                            nc.tensor.matmul(
                                ps, wsb[:, ec, 128 * dt_:128 * dt_ + 128],
                                xt_sb[:, ec, 512 * sh:512 * sh + 512],
                                start=(ec == 0), stop=(ec == EC - 1),
                            )
                        nc.vector.tensor_scalar_add(
                            out=dest[:, dt_, 512 * sh:512 * sh + 512],
                            in0=ps, scalar1=bias[:, dt_:dt_ + 1],
                        )
            for tt in range(TT):
                ps = mm.tile([128, 512], f32, tag="mm")
                for ec in range(EC):
                    nc.tensor.matmul(
                        ps, xt_sb[:, ec, 128 * tt:128 * tt + 128],
                        wv_sb[:, ec, :],
                        start=(ec == 0), stop=(ec == EC - 1),
                    )
                nc.vector.tensor_copy(
                    vview[:, tt, :, 0, :], ps.rearrange("p (h d) -> p h d", d=DH)
                )
            p1.release()
            late = tc.alloc_tile_pool(name="late", bufs=1)
            expa = tc.alloc_tile_pool(name="expa", bufs=8)
            expb = tc.alloc_tile_pool(name="expb", bufs=8)
            small = tc.alloc_tile_pool(name="small", bufs=2)
            outp = tc.alloc_tile_pool(name="outp", bufs=2)
            wo_sb = late.tile([128, EC, 1024], f32r)
            nc.sync.dma_start(out=wo_sb, in_=wo.ap())

            for hp in range(HPC // 2):
                pair = (2 * hp, 2 * hp + 1)
                et = {}
                for tt in range(TT):
                    js = [j for j in range(NJ) if computed(tt, j)]
                    s0 = 512 * js[0]
                    c0 = 128 * tt
                    pss = {}
                    for h in pair:
                        dt_ = h // 2
                        pb = 64 * (h % 2)
                        ps = mm.tile([128, 1024], f32, tag="mm", name=f"ps_{h}_{tt}")
                        pss[h] = ps
                        for j in js:
                            lo = max(512 * j, c0) if causal else 512 * j
                            nc.tensor.matmul(
                                ps[:, lo - s0:512 * j + 512 - s0],
                                kt_sb[pb:pb + 64, dt_, c0:c0 + 128],
                                qt_sb[pb:pb + 64, dt_, lo:512 * j + 512],
                                start=True, stop=True,
                            )
                            if not causal:
                                o = 512 * j - s0
                                nc.vector.tensor_add(
                                    ps[:, o:o + 512],
                                    ps[:, o:o + 512],
                                    mkt_sb[:, tt, 512 * j:512 * j + 512],
                                )
                    for h in pair:
                        ps = pss[h]
                        if tt < 4 or not causal:
                            e = expa.tile([128, 1024], f32r, tag="expa",
                                          name=f"e_{h}_{tt}")
                        else:
                            e = expb.tile([128, 512], f32r, tag="expb",
                                          name=f"e_{h}_{tt}")
                        if causal:
                            nc.scalar.activation(
                                e[:, c0 - s0:], ps[:, c0 - s0:1024 - s0],
                                Exp, scale=0.5,
                            )
                            nc.gpsimd.affine_select(
                                out=e[:, 0:c0 + 128 - s0], in_=e[:, 0:c0 + 128 - s0],
                                pattern=[[1, c0 + 128 - s0]], compare_op=is_ge,
                                fill=0.0, base=s0 - c0, channel_multiplier=-1,
                            )
                        else:
                            nc.scalar.activation(
                                e[:, :], ps[:, :1024 - s0], Exp, scale=0.5
                            )
                        for j in js:
                            et[(h, tt, j)] = e[:, 512 * j - s0:512 * j - s0 + 512]
                for h in pair:
                    zt_f = small.tile([64, S], f32, tag="ztf", name=f"ztf_{h}")
                    rec = small.tile([64, S], f32, tag="rec", name=f"rec_{h}")
                    for j in range(NJ):
                        zt = ztp.tile([128, 512], f32, tag="zt", name=f"zt_{h}_{j}")
                        ks = [tt for tt in range(TT) if (h, tt, j) in et]
                        for i, tt in enumerate(ks):
                            lo = max(0, 128 * tt - 512 * j) if causal else 0
                            nc.tensor.matmul(
                                zt[:, lo:], vp_sb[:, tt, 128 * h:128 * h + 128],
                                et[(h, tt, j)][:, lo:],
                                start=(i == 0), stop=(i == len(ks) - 1),
                            )
                        nc.vector.reciprocal(rec[:, 512 * j:512 * j + 512],
                                             zt[64:128, :])
                        nc.vector.tensor_copy(zt_f[:, 512 * j:512 * j + 512],
                                              zt[0:64, :])
                    zv = zt_f.rearrange("p (m c par) -> par p c m", m=64, c=8, par=2)
                    rv = rec.rearrange("p (m c par) -> par p c m", m=64, c=8, par=2)
                    for P in range(2):
                        nc.vector.tensor_tensor(
                            x2t_sb[64 * P:64 * P + 64, :, 64 * h:64 * h + 64],
                            zv[P], rv[P], op=mult,
                        )

            for tp in range(4):
                osb = outp.tile([128, 1024], f32, tag="osb")
                for eh in range(2):
                    ps = mm.tile([128, 512], f32, tag="mm")
                    for c in range(EC):
                        nc.tensor.matmul(
                            ps, x2t_sb[:, c, 128 * tp:128 * tp + 128],
                            wo_sb[:, c, 512 * eh:512 * eh + 512],
                            start=(c == 0), stop=(c == EC - 1),
                        )
                    nc.vector.tensor_copy(osb[:, 512 * eh:512 * eh + 512], ps)
                nc.sync.dma_start(out=out[tp], in_=osb)
            for p in (outp, small, expb, expa, late):
                p.release()
    nc.compile()
    return nc


def kernel(inputs, mask, wq, bq, wk, bk, wv, bv, wo, bo):
    from concourse.bass_utils import run_bass_kernel_spmd

    x = np.asarray(inputs, dtype=np.float32)
    wq = np.asarray(wq, dtype=np.float32)
    wk = np.asarray(wk, dtype=np.float32)
    wv = np.asarray(wv, dtype=np.float32)
    wo = np.asarray(wo, dtype=np.float32)
    bq = np.asarray(bq, dtype=np.float32)
    bk = np.asarray(bk, dtype=np.float32)
    mask2d = np.asarray(mask, dtype=np.float32).reshape(S, S)
    causal_ref = 1.0 - np.tril(np.ones((S, S), dtype=np.float32))
    causal = bool(np.array_equal(mask2d, causal_ref))
    fast = causal and not np.any(bq != 0) and not np.any(bk != 0)

    global _last_variant
    if fast:
        _last_variant = "fast"
        if "fast" not in _CACHE:
            _CACHE["fast"] = _build_fast()
        nc = _CACHE["fast"]
        woh = np.ascontiguousarray(
            wo.reshape(EC, 128, 1024).transpose(1, 0, 2)).astype(np.float16)
        xts = [np.ascontiguousarray(
            x[b].T.reshape(EC, 128, S).transpose(1, 0, 2)).astype(np.float16)
            for b in range(B)]
        whs = {}
        for name, w in (("wq", wq), ("wk", wk), ("wv", wv)):
            whs[name] = [np.ascontiguousarray(
                w[:, 512 * hg:512 * hg + 512].reshape(EC, 128, 512)
                .transpose(1, 0, 2)).astype(np.float16) for hg in range(2)]
        in_maps = []
        for c in range(NCORES):
            b, hg = c // 2, c % 2
            in_maps.append({
                "xt": xts[b],
                "wq": whs["wq"][hg],
                "wk": whs["wk"][hg],
                "wv": whs["wv"][hg],
                "wo": woh,
            })
    else:
        variant = "causal" if causal else "generic"
        _last_variant = variant
        if variant not in _CACHE:
            _CACHE[variant] = _build_legacy(variant)
        nc = _CACHE[variant]
        in_maps = []
        for c in range(NCORES):
            b, hg = c // 2, c % 2
            sl = slice(512 * hg, 512 * hg + 512)
            m = {
                "xt": np.ascontiguousarray(x[b].T.reshape(EC, 128, S).transpose(1, 0, 2)),
                "wq": np.ascontiguousarray(wq[:, sl].reshape(EC, 128, 512).transpose(1, 0, 2)),
                "wk": np.ascontiguousarray(wk[:, sl].reshape(EC, 128, 512).transpose(1, 0, 2)),
                "wv": np.ascontiguousarray(wv[:, sl].reshape(EC, 128, 512).transpose(1, 0, 2)),
                "wo": np.ascontiguousarray(wo.reshape(EC, 128, 1024).transpose(1, 0, 2)),
                "bq": np.ascontiguousarray(bq[sl].reshape(4, 128).T),
                "bk": np.ascontiguousarray(bk[sl].reshape(4, 128).T),
            }
            if not causal:
                m["mkt"] = np.ascontiguousarray(
                    (mask2d.T * np.float32(-2e9)).reshape(TT, 128, S).transpose(1, 0, 2))
            in_maps.append(m)

    res = run_bass_kernel_spmd(nc, in_maps, core_ids=list(range(NCORES)))
    full = np.empty((B, S, E), dtype=np.float32)
    for c in range(NCORES):
        b, hg = c // 2, c % 2
        full[b, 512 * hg:512 * hg + 512, :] = res.results[c]["out"].reshape(512, 1024)

    # biases bv/bo are zero in this problem; fold in exactly if ever nonzero.
    bv = np.asarray(bv, dtype=np.float32)
    bo = np.asarray(bo, dtype=np.float32)
    if np.any(bv != 0):
        # z_norm[b,h,s,d] += bv[64h+d]  =>  X2 += Bmat  =>  out += Bmat @ wo
        bmat = np.zeros((S, E), dtype=np.float64)
        tpr = np.arange(S)
        e = np.arange(E)
        bmat[:, :] = bv[(64 * (tpr[:, None] // 64) + e[None, :] % 64)]
        full += (bmat @ np.asarray(wo, dtype=np.float64)).astype(np.float32)[None]
    if np.any(bo != 0):
        full += bo[None, None, :]
    return full


# revision 82
# speedup vs baseline: 1.0072x; 1.0072x over previous
"""Trainium2 Bass kernel for MultiHeadAttention (B=4, S=1024, E=1024, H=16, Dh=64).

Sharding: 8 cores = (batch b in 0..3) x (head-group hg in 0..1, 8 heads each).
The reference reshapes [B,H,S,Dh] -> [B,S,E] WITHOUT transposing heads back, so
head h's attention output occupies output rows t' = h*64 + s//16 — the final
projection is row-parallel across head groups: no cross-core communication.

Fast path (causal mask, zero bq/bk — the graded configuration), ~100us/core
(from the 136us fp32r baseline):
  All matmuls fp16 (same PE rate as fp32r, no narrow-width penalty, half the
  DMA/SBUF, better mantissa) except attn@V which is bf16 (exp outputs reach
  ~1e13, beyond fp16 range).  exp(0.5*x) on ACT writes bf16 from per-512-col
  PSUM score tiles; causal fill via a 128-wide gpsimd affine_select on just
  the diagonal block.  The PE stream is hand-interleaved (score groups spread
  between K/V-projection, attn@V and out-projection units) so the
  scores->exp->attn@V chain pipelines against independent matmul work, and
  is led by dummy warmup matmuls + split-K first Q groups so the array is at
  the top p-state and busy as soon as the first weight DMA chunks land.
  Normalization (1/rowsum) reads z straight from PSUM and fuses the
  scrambled-reshape gather, writing fp16 into the out-projection stationary.
  Deep pools (4 score PSUM tiles, 24 e-tiles, 4 output staging tiles) keep
  every producer/consumer WAR chain off the critical path; the epilogue
  hides the last head's normalize under an earlier output tile and splits
  the final DMA so the post-matmul tail is ~3.5us.

Fallback (non-causal mask or nonzero bq/bk): original fp32r implementation.
"""
import numpy as np

B, S, E, H, DH = 4, 1024, 1024, 16, 64
NCORES = 8
HPC = 8          # heads per core
EC = 8           # 128-row chunks of E
TT = 8           # 128-row t-tiles of S
NJ = 2           # 512-col s-blocks

_CACHE = {}


def _computed(tt, j):
    # scores^T block (t-tile tt, s-block j) skipped iff fully masked (t > s)
    return 128 * tt <= 512 * j + 511


def _build_fast():
    import concourse.bacc as bacc
    import concourse.tile as tile
    import concourse.mybir as mybir

    f32 = mybir.dt.float32
    f16 = mybir.dt.float16
    bf16 = mybir.dt.bfloat16
    Exp = mybir.ActivationFunctionType.Exp
    Copy = mybir.ActivationFunctionType.Copy
    mult = mybir.AluOpType.mult
    is_ge = mybir.AluOpType.is_ge

    nc = bacc.Bacc("TRN2")
    xt = nc.dram_tensor("xt", [128, EC, S], f16, kind="ExternalInput")
    wq = nc.dram_tensor("wq", [128, EC, 512], f16, kind="ExternalInput")
    wk = nc.dram_tensor("wk", [128, EC, 512], f16, kind="ExternalInput")
    wv = nc.dram_tensor("wv", [128, EC, 512], f16, kind="ExternalInput")
    wo = nc.dram_tensor("wo", [128, EC, 1024], f16, kind="ExternalInput")
    out = nc.dram_tensor("out", [4, 128, 1024], f32, kind="ExternalOutput")

    with tile.TileContext(nc) as tc:
        with (
            tc.tile_pool(name="pers", bufs=1) as pp,
            tc.tile_pool(name="pj", bufs=2, space="PSUM") as pj,
            tc.tile_pool(name="sc", bufs=4, space="PSUM") as sc,
            tc.tile_pool(name="ztp", bufs=2, space="PSUM") as ztp,
            tc.tile_pool(name="expa", bufs=24) as expa,
            tc.tile_pool(name="expb", bufs=24) as expb,
            tc.tile_pool(name="small", bufs=2) as small,
            tc.tile_pool(name="outp", bufs=6) as outp,
        ):
            # NOTE: sc tiles are [128,1024] (2 PSUM banks): pj(2)+sc(2x2)+ztp(2)=8
            xt_sb = pp.tile([128, EC, S], f16)
            wq_sb = pp.tile([128, EC, 512], f16)
            wk_sb = pp.tile([128, EC, 512], f16)
            wv_sb = pp.tile([128, EC, 512], f16)
            wo_sb = pp.tile([128, EC, 1024], f16)
            qt_sb = pp.tile([128, 4, S], f16)
            kt_sb = pp.tile([128, 4, S], f16)
            vp_sb = pp.tile([128, TT, 1024], bf16)
            x2t_sb = pp.tile([128, EC, 512], f16)
            dummy = pp.tile([128, 640], f16)

            # DMA order: Q-projection deps first (half-K chunks so the split-K
            # Q groups can start early), then stream the rest.
            for k0, k1 in ((0, 2), (2, 4), (4, 8)):
                nc.sync.dma_start(out=wq_sb[:, k0:k1, :], in_=wq[:, k0:k1, :])
                nc.sync.dma_start(out=xt_sb[:, k0:k1, 0:512],
                                  in_=xt[:, k0:k1, 0:512])
            nc.sync.dma_start(out=xt_sb[:, :, 512:1024], in_=xt[:, :, 512:1024])
            nc.sync.dma_start(out=wk_sb, in_=wk.ap())
            nc.sync.dma_start(out=wv_sb, in_=wv.ap())
            nc.sync.dma_start(out=wo_sb, in_=wo.ap())

            nc.gpsimd.memset(dummy, 0.0)
            # ones everywhere; V-projection copies overwrite the value halves
            nc.gpsimd.memset(vp_sb, 1.0)
            vview = vp_sb.rearrange("p t (h two d) -> p t h two d", two=2, d=DH)

            # ---- PE p-state warmup: keep the array busy during input DMA ----
            for i in range(8):
                ps = pj.tile([128, 512], f32, tag="pj", name=f"warm_{i}")
                nc.tensor.matmul(ps, dummy[:, 0:128], dummy[:, 128:640],
                                 start=True, stop=True)

            # ---- Q^T projection: [d-tile, s] = wq.T @ X^T  (all of it) ----
            def proj_qk(wsb, dest, dt_, sh, on_act=False):
                ps = pj.tile([128, 512], f32, tag="pj")
                for ec in range(EC):
                    nc.tensor.matmul(
                        ps, wsb[:, ec, 128 * dt_:128 * dt_ + 128],
                        xt_sb[:, ec, 512 * sh:512 * sh + 512],
                        start=(ec == 0), stop=(ec == EC - 1),
                    )
                if on_act:
                    # ACT is idle before the first exps; avoids queueing the
                    # kt copy behind V-projection copies on DVE
                    nc.scalar.activation(
                        dest[:, dt_, 512 * sh:512 * sh + 512], ps, Copy)
                else:
                    nc.vector.tensor_copy(
                        dest[:, dt_, 512 * sh:512 * sh + 512], ps)

            # Q sh0 via split-K: accumulation groups stay open across 4 passes
            # so each 2-ec chunk is consumed as soon as its DMA lands.
            # The 4 tiles live in the (still idle) scores pool.
            psq = {}
            for dt_ in range(4):
                psq[dt_] = sc.tile([128, 512], f32, tag="sc", name=f"q0_{dt_}")
            for k0, k1 in ((0, 2), (2, 4), (4, 8)):
                for dt_ in range(4):
                    for ec in range(k0, k1):
                        nc.tensor.matmul(
                            psq[dt_], wq_sb[:, ec, 128 * dt_:128 * dt_ + 128],
                            xt_sb[:, ec, 0:512],
                            start=(ec == 0), stop=(ec == 7),
                        )
            for dt_ in range(4):
                nc.vector.tensor_copy(qt_sb[:, dt_, 0:512], psq[dt_])
            for dt_ in range(4):
                proj_qk(wq_sb, qt_sb, dt_, 1)
            # K dt0 (both s-halves): unblocks pair-0 scores early
            proj_qk(wk_sb, kt_sb, 0, 0)
            proj_qk(wk_sb, kt_sb, 0, 1)

            def proj_v(tt):
                ps = pj.tile([128, 512], f32, tag="pj")
                for ec in range(EC):
                    nc.tensor.matmul(
                        ps, xt_sb[:, ec, 128 * tt:128 * tt + 128],
                        wv_sb[:, ec, :],
                        start=(ec == 0), stop=(ec == EC - 1),
                    )
                nc.vector.tensor_copy(
                    vview[:, tt, :, 0, :], ps.rearrange("p (h d) -> p h d", d=DH)
                )

            et = {}

            def scores_tt(hp, tt):
                # emit scores matmuls + exp (+ causal fill) for head pair hp,
                # t-tile tt; per-j [128,512] PSUM tiles for pipeline depth.
                pair = (2 * hp, 2 * hp + 1)
                c0 = 128 * tt
                js = [j for j in range(NJ) if _computed(tt, j)]
                s0 = 512 * js[0]
                es = {}
                for h in pair:
                    if tt < 4:
                        es[h] = expa.tile([128, 1024], bf16, tag="expa",
                                          name=f"e_{h}_{tt}")
                    else:
                        es[h] = expb.tile([128, 512], bf16, tag="expb",
                                          name=f"e_{h}_{tt}")
                for j in js:
                    for h in pair:
                        dt_ = h // 2
                        pb = 64 * (h % 2)
                        e = es[h]
                        lo = max(512 * j, c0)
                        ps = sc.tile([128, 512], f32, tag="sc",
                                     name=f"ps_{h}_{tt}_{j}")
                        nc.tensor.matmul(
                            ps[:, lo - 512 * j:512],
                            kt_sb[pb:pb + 64, dt_, c0:c0 + 128],
                            qt_sb[pb:pb + 64, dt_, lo:512 * j + 512],
                            start=True, stop=True,
                        )
                        # e columns are true s minus s0
                        nc.scalar.activation(
                            e[:, lo - s0:512 * j + 512 - s0],
                            ps[:, lo - 512 * j:512], Exp, scale=0.5,
                        )
                        if lo == c0:
                            # zero the below-diagonal triangle (only this
                            # 128-col diagonal block is read with cols < t)
                            nc.gpsimd.affine_select(
                                out=e[:, c0 - s0:c0 + 128 - s0],
                                in_=e[:, c0 - s0:c0 + 128 - s0],
                                pattern=[[1, 128]], compare_op=is_ge,
                                fill=0.0, base=0, channel_multiplier=-1,
                            )
                        et[(h, tt, j)] = e[:, 512 * j - s0:512 * j - s0 + 512]

            recs = {}

            def z_unit(h, j):
                # attn @ V for head h, s-block j (bf16), then 1/sum + scrambled
                # normalize directly from PSUM into x2t (out-proj stationary)
                zt = ztp.tile([128, 512], f32, tag="zt", name=f"zt_{h}_{j}")
                ks = [tt for tt in range(TT) if (h, tt, j) in et]
                for i, tt in enumerate(ks):
                    lo = max(0, 128 * tt - 512 * j)
                    nc.tensor.matmul(
                        zt[:, lo:], vp_sb[:, tt, 128 * h:128 * h + 128],
                        et[(h, tt, j)][:, lo:],
                        start=(i == 0), stop=(i == len(ks) - 1),
                    )
                if h not in recs:
                    recs[h] = small.tile([64, S], f32, tag="rec", name=f"rec_{h}")
                rec = recs[h]
                nc.vector.reciprocal(rec[:, 512 * j:512 * j + 512], zt[64:128, :])
                zv = zt[0:64].rearrange("p (m c par) -> par p c m", m=32, c=8, par=2)
                rv = rec[:, 512 * j:512 * j + 512].rearrange(
                    "p (m c par) -> par p c m", m=32, c=8, par=2)
                for P in range(2):
                    nc.vector.tensor_tensor(
                        x2t_sb[64 * P:64 * P + 64, :,
                               64 * h + 32 * j:64 * h + 32 * j + 32],
                        zv[P], rv[P], op=mult,
                    )

            def out_unit(tp, eh, split=1):
                ps = pj.tile([128, 512], f32, tag="pj")
                for c in range(EC):
                    nc.tensor.matmul(
                        ps, x2t_sb[:, c, 128 * tp:128 * tp + 128],
                        wo_sb[:, c, 512 * eh:512 * eh + 512],
                        start=(c == 0), stop=(c == EC - 1),
                    )
                osb = outp.tile([128, 512], f32, tag="osb")
                # split>1 pipelines copy+DMA chunks to shorten the final tail
                w = 512 // split
                for i in range(split):
                    nc.vector.tensor_copy(osb[:, i * w:(i + 1) * w],
                                          ps[:, i * w:(i + 1) * w])
                    nc.sync.dma_start(
                        out=out[tp][:, 512 * eh + i * w:512 * eh + (i + 1) * w],
                        in_=osb[:, i * w:(i + 1) * w])

            # ---- interleaved PE stream ----
            # Each phase emits pair-hp's 8 score groups ("s", tt) spaced among
            # fill work; fills lead each phase so the previous pair's exps can
            # drain (scores PSUM bufs WAR against them).
            phases = [
                # pair-0 scores x {K-proj dt1..3, V-proj tt0,1}
                (0, [("k", 1, 0), "s", ("k", 1, 1), "s", ("k", 2, 0), "s",
                     ("k", 2, 1), "s", ("k", 3, 0), "s", ("k", 3, 1), "s",
                     ("v", 0), "s", ("v", 1), "s"]),
                # pair-1 scores x {V-proj tt2..7, z h0, z h1 j0}
                (1, [("v", 2), "s", ("v", 3), "s", ("v", 4), "s", ("v", 5),
                     "s", ("v", 6), "s", ("v", 7), "s", ("z", 0, 0), "s",
                     ("z", 0, 1), "s", ("z", 1, 0)]),
                # pair-2 scores x {z h1 j1..h3, out0}
                (2, [("z", 1, 1), "s", ("o", 0, 0), "s",
                     ("o", 0, 1), "s", ("z", 2, 0), "s", ("z", 2, 1), "s",
                     ("z", 3, 0), "s", "s", ("z", 3, 1), "s"]),
                # pair-3 scores x {z h4..h5, z h6 j0, out1}
                (3, [("o", 1, 0), ("o", 1, 1), "s", ("z", 4, 0), "s",
                     ("z", 4, 1), "s", ("z", 5, 0), "s", ("z", 5, 1), "s",
                     ("z", 6, 0), "s", "s", "s"]),
            ]

            def emit(u):
                if u[0] == "k":
                    proj_qk(wk_sb, kt_sb, u[1], u[2], on_act=True)
                elif u[0] == "v":
                    proj_v(u[1])
                elif u[0] == "z":
                    z_unit(u[1], u[2])
                else:
                    out_unit(u[1], u[2])

            for hp, seq in phases:
                tt = 0
                for u in seq:
                    if u == "s":
                        scores_tt(hp, tt)
                        tt += 1
                    else:
                        emit(u)
                assert tt == TT

            # tail: j0 z-units first (their exps drained long ago), j1 units
            # as the last pair-3 exps land; out2 hides h7's normalize.
            for u in (("z", 7, 0), ("z", 6, 1), ("z", 7, 1),
                      ("o", 2, 0), ("o", 2, 1), ("o", 3, 0)):
                emit(u)
            # final half-tile: two independent 256-col groups on the idle z
            # pool so the first chunk's DMA chain starts before the last mms
            for i in range(2):
                ps = ztp.tile([128, 256], f32, tag="zt")
                for c in range(EC):
                    nc.tensor.matmul(
                        ps, x2t_sb[:, c, 384:512],
                        wo_sb[:, c, 512 + 256 * i:768 + 256 * i],
                        start=(c == 0), stop=(c == EC - 1),
                    )
                osb = outp.tile([128, 256], f32, tag="osb")
                nc.vector.tensor_copy(osb, ps)
                nc.sync.dma_start(out=out[3][:, 512 + 256 * i:768 + 256 * i],
                                  in_=osb)
    nc.compile()
    return nc


def _build_legacy(variant):
    import concourse.bacc as bacc
    import concourse.tile as tile
    import concourse.mybir as mybir

    f32 = mybir.dt.float32
    f32r = mybir.dt.float32r
    Exp = mybir.ActivationFunctionType.Exp
    mult = mybir.AluOpType.mult
    is_ge = mybir.AluOpType.is_ge

    causal = variant == "causal"

    def computed(tt, j):
        if not causal:
            return True
        return 128 * tt <= 512 * j + 511

    nc = bacc.Bacc("TRN2")
    xt = nc.dram_tensor("xt", [128, EC, S], f32r, kind="ExternalInput")
    wq = nc.dram_tensor("wq", [128, EC, 512], f32r, kind="ExternalInput")
    wk = nc.dram_tensor("wk", [128, EC, 512], f32r, kind="ExternalInput")
    wv = nc.dram_tensor("wv", [128, EC, 512], f32r, kind="ExternalInput")
    wo = nc.dram_tensor("wo", [128, EC, 1024], f32r, kind="ExternalInput")
    bq = nc.dram_tensor("bq", [128, 4], f32, kind="ExternalInput")
    bk = nc.dram_tensor("bk", [128, 4], f32, kind="ExternalInput")
    if not causal:
        mkt = nc.dram_tensor("mkt", [128, TT, S], f32, kind="ExternalInput")
    out = nc.dram_tensor("out", [4, 128, 1024], f32, kind="ExternalOutput")

    with tile.TileContext(nc) as tc:
        with (
            tc.tile_pool(name="persist", bufs=1) as pp,
            tc.tile_pool(name="mm", bufs=3, space="PSUM") as mm,
            tc.tile_pool(name="ztp", bufs=2, space="PSUM") as ztp,
        ):
            p1 = tc.alloc_tile_pool(name="p1", bufs=1)
            xt_sb = p1.tile([128, EC, S], f32r)
            wq_sb = p1.tile([128, EC, 512], f32r)
            wk_sb = p1.tile([128, EC, 512], f32r)
            wv_sb = p1.tile([128, EC, 512], f32r)
            for k in range(0, EC, 2):
                nc.sync.dma_start(out=xt_sb[:, k:k + 2, :], in_=xt[:, k:k + 2, :])
                nc.sync.dma_start(out=wq_sb[:, k:k + 2, :], in_=wq[:, k:k + 2, :])
                nc.sync.dma_start(out=wk_sb[:, k:k + 2, :], in_=wk[:, k:k + 2, :])
                nc.sync.dma_start(out=wv_sb[:, k:k + 2, :], in_=wv[:, k:k + 2, :])
            qt_sb = pp.tile([128, 4, S], f32r)
            kt_sb = pp.tile([128, 4, S], f32r)
            vp_sb = pp.tile([128, TT, 1024], f32r)
            x2t_sb = pp.tile([128, EC, 512], f32r)
            bq_sb = pp.tile([128, 4], f32)
            bk_sb = pp.tile([128, 4], f32)
            if not causal:
                mkt_sb = pp.tile([128, TT, S], f32)
                nc.sync.dma_start(out=mkt_sb, in_=mkt.ap())
            nc.sync.dma_start(out=bq_sb, in_=bq.ap())
            nc.sync.dma_start(out=bk_sb, in_=bk.ap())

            vview = vp_sb.rearrange("p t (h two d) -> p t h two d", two=2, d=DH)
            ones_sb = pp.tile([128, 512], f32)
            nc.vector.memset(ones_sb, 1.0)
            ones_v = ones_sb.rearrange("p (h d) -> p h d", d=DH)
            for tt in range(TT):
                nc.vector.tensor_copy(vview[:, tt, :, 1, :], ones_v)

            for wsb, dest, bias in ((wq_sb, qt_sb, bq_sb), (wk_sb, kt_sb, bk_sb)):
                for dt_ in range(4):
                    for sh in range(2):
                        ps = mm.tile([128, 512], f32, tag="mm")
                        for ec in range(EC):
                            nc.tensor.matmul(
                                ps, wsb[:, ec, 128 * dt_:128 * dt_ + 128],
                                xt_sb[:, ec, 512 * sh:512 * sh + 512],
                                start=(ec == 0), stop=(ec == EC - 1),
                            )
                        nc.vector.tensor_scalar_add(
                            out=dest[:, dt_, 512 * sh:512 * sh + 512],
                            in0=ps, scalar1=bias[:, dt_:dt_ + 1],
                        )
            for tt in range(TT):
                ps = mm.tile([128, 512], f32, tag="mm")
                for ec in range(EC):
                    nc.tensor.matmul(
                        ps, xt_sb[:, ec, 128 * tt:128 * tt + 128],
                        wv_sb[:, ec, :],
                        start=(ec == 0), stop=(ec == EC - 1),
                    )
                nc.vector.tensor_copy(
                    vview[:, tt, :, 0, :], ps.rearrange("p (h d) -> p h d", d=DH)
                )
            p1.release()
            late = tc.alloc_tile_pool(name="late", bufs=1)
            expa = tc.alloc_tile_pool(name="expa", bufs=8)
            expb = tc.alloc_tile_pool(name="expb", bufs=8)
            small = tc.alloc_tile_pool(name="small", bufs=2)
            outp = tc.alloc_tile_pool(name="outp", bufs=2)
            wo_sb = late.tile([128, EC, 1024], f32r)
            nc.sync.dma_start(out=wo_sb, in_=wo.ap())

            for hp in range(HPC // 2):
                pair = (2 * hp, 2 * hp + 1)
                et = {}
                for tt in range(TT):
                    js = [j for j in range(NJ) if computed(tt, j)]
                    s0 = 512 * js[0]
                    c0 = 128 * tt
                    pss = {}
                    for h in pair:
                        dt_ = h // 2
                        pb = 64 * (h % 2)
                        ps = mm.tile([128, 1024], f32, tag="mm", name=f"ps_{h}_{tt}")
                        pss[h] = ps
                        for j in js:
                            lo = max(512 * j, c0) if causal else 512 * j
                            nc.tensor.matmul(
                                ps[:, lo - s0:512 * j + 512 - s0],
                                kt_sb[pb:pb + 64, dt_, c0:c0 + 128],
                                qt_sb[pb:pb + 64, dt_, lo:512 * j + 512],
                                start=True, stop=True,
                            )
                            if not causal:
                                o = 512 * j - s0
                                nc.vector.tensor_add(
                                    ps[:, o:o + 512],
                                    ps[:, o:o + 512],
                                    mkt_sb[:, tt, 512 * j:512 * j + 512],
                                )
                    for h in pair:
                        ps = pss[h]
                        if tt < 4 or not causal:
                            e = expa.tile([128, 1024], f32r, tag="expa",
                                          name=f"e_{h}_{tt}")
                        else:
                            e = expb.tile([128, 512], f32r, tag="expb",
                                          name=f"e_{h}_{tt}")
                        if causal:
                            nc.scalar.activation(
                                e[:, c0 - s0:], ps[:, c0 - s0:1024 - s0],
                                Exp, scale=0.5,
                            )
                            nc.gpsimd.affine_select(
                                out=e[:, 0:c0 + 128 - s0], in_=e[:, 0:c0 + 128 - s0],
                                pattern=[[1, c0 + 128 - s0]], compare_op=is_ge,
                                fill=0.0, base=s0 - c0, channel_multiplier=-1,
                            )
                        else:
                            nc.scalar.activation(
                                e[:, :], ps[:, :1024 - s0], Exp, scale=0.5
                            )
                        for j in js:
                            et[(h, tt, j)] = e[:, 512 * j - s0:512 * j - s0 + 512]
                for h in pair:
                    zt_f = small.tile([64, S], f32, tag="ztf", name=f"ztf_{h}")
                    rec = small.tile([64, S], f32, tag="rec", name=f"rec_{h}")
                    for j in range(NJ):
                        zt = ztp.tile([128, 512], f32, tag="zt", name=f"zt_{h}_{j}")
                        ks = [tt for tt in range(TT) if (h, tt, j) in et]
                        for i, tt in enumerate(ks):
                            lo = max(0, 128 * tt - 512 * j) if causal else 0
                            nc.tensor.matmul(
                                zt[:, lo:], vp_sb[:, tt, 128 * h:128 * h + 128],
                                et[(h, tt, j)][:, lo:],
                                start=(i == 0), stop=(i == len(ks) - 1),
                            )
                        nc.vector.reciprocal(rec[:, 512 * j:512 * j + 512],
                                             zt[64:128, :])
                        nc.vector.tensor_copy(zt_f[:, 512 * j:512 * j + 512],
                                              zt[0:64, :])
                    zv = zt_f.rearrange("p (m c par) -> par p c m", m=64, c=8, par=2)
                    rv = rec.rearrange("p (m c par) -> par p c m", m=64, c=8, par=2)
                    for P in range(2):
                        nc.vector.tensor_tensor(
                            x2t_sb[64 * P:64 * P + 64, :, 64 * h:64 * h + 64],
                            zv[P], rv[P], op=mult,
                        )

            for tp in range(4):
                osb = outp.tile([128, 1024], f32, tag="osb")
                for eh in range(2):
                    ps = mm.tile([128, 512], f32, tag="mm")
                    for c in range(EC):
                        nc.tensor.matmul(
                            ps, x2t_sb[:, c, 128 * tp:128 * tp + 128],
                            wo_sb[:, c, 512 * eh:512 * eh + 512],
                            start=(c == 0), stop=(c == EC - 1),
                        )
                    nc.vector.tensor_copy(osb[:, 512 * eh:512 * eh + 512], ps)
                nc.sync.dma_start(out=out[tp], in_=osb)
            for p in (outp, small, expb, expa, late):
                p.release()
    nc.compile()
    return nc


def kernel(inputs, mask, wq, bq, wk, bk, wv, bv, wo, bo):
    from concourse.bass_utils import run_bass_kernel_spmd

    x = np.asarray(inputs, dtype=np.float32)
    wq = np.asarray(wq, dtype=np.float32)
    wk = np.asarray(wk, dtype=np.float32)
    wv = np.asarray(wv, dtype=np.float32)
    wo = np.asarray(wo, dtype=np.float32)
    bq = np.asarray(bq, dtype=np.float32)
    bk = np.asarray(bk, dtype=np.float32)
    mask2d = np.asarray(mask, dtype=np.float32).reshape(S, S)
    causal_ref = 1.0 - np.tril(np.ones((S, S), dtype=np.float32))
    causal = bool(np.array_equal(mask2d, causal_ref))
    fast = causal and not np.any(bq != 0) and not np.any(bk != 0)

    global _last_variant
    if fast:
        _last_variant = "fast"
        if "fast" not in _CACHE:
            _CACHE["fast"] = _build_fast()
        nc = _CACHE["fast"]
        woh = np.ascontiguousarray(
            wo.reshape(EC, 128, 1024).transpose(1, 0, 2)).astype(np.float16)
        xts = [np.ascontiguousarray(
            x[b].T.reshape(EC, 128, S).transpose(1, 0, 2)).astype(np.float16)
            for b in range(B)]
        whs = {}
        for name, w in (("wq", wq), ("wk", wk), ("wv", wv)):
            whs[name] = [np.ascontiguousarray(
                w[:, 512 * hg:512 * hg + 512].reshape(EC, 128, 512)
                .transpose(1, 0, 2)).astype(np.float16) for hg in range(2)]
        in_maps = []
        for c in range(NCORES):
            b, hg = c // 2, c % 2
            in_maps.append({
                "xt": xts[b],
                "wq": whs["wq"][hg],
                "wk": whs["wk"][hg],
                "wv": whs["wv"][hg],
                "wo": woh,
            })
    else:
        variant = "causal" if causal else "generic"
        _last_variant = variant
        if variant not in _CACHE:
            _CACHE[variant] = _build_legacy(variant)
        nc = _CACHE[variant]
        in_maps = []
        for c in range(NCORES):
            b, hg = c // 2, c % 2
            sl = slice(512 * hg, 512 * hg + 512)
            m = {
                "xt": np.ascontiguousarray(x[b].T.reshape(EC, 128, S).transpose(1, 0, 2)),
                "wq": np.ascontiguousarray(wq[:, sl].reshape(EC, 128, 512).transpose(1, 0, 2)),
                "wk": np.ascontiguousarray(wk[:, sl].reshape(EC, 128, 512).transpose(1, 0, 2)),
                "wv": np.ascontiguousarray(wv[:, sl].reshape(EC, 128, 512).transpose(1, 0, 2)),
                "wo": np.ascontiguousarray(wo.reshape(EC, 128, 1024).transpose(1, 0, 2)),
                "bq": np.ascontiguousarray(bq[sl].reshape(4, 128).T),
                "bk": np.ascontiguousarray(bk[sl].reshape(4, 128).T),
            }
            if not causal:
                m["mkt"] = np.ascontiguousarray(
                    (mask2d.T * np.float32(-2e9)).reshape(TT, 128, S).transpose(1, 0, 2))
            in_maps.append(m)

    res = run_bass_kernel_spmd(nc, in_maps, core_ids=list(range(NCORES)))
    full = np.empty((B, S, E), dtype=np.float32)
    for c in range(NCORES):
        b, hg = c // 2, c % 2
        full[b, 512 * hg:512 * hg + 512, :] = res.results[c]["out"].reshape(512, 1024)

    # biases bv/bo are zero in this problem; fold in exactly if ever nonzero.
    bv = np.asarray(bv, dtype=np.float32)
    bo = np.asarray(bo, dtype=np.float32)
    if np.any(bv != 0):
        # z_norm[b,h,s,d] += bv[64h+d]  =>  X2 += Bmat  =>  out += Bmat @ wo
        bmat = np.zeros((S, E), dtype=np.float64)
        tpr = np.arange(S)
        e = np.arange(E)
        bmat[:, :] = bv[(64 * (tpr[:, None] // 64) + e[None, :] % 64)]
        full += (bmat @ np.asarray(wo, dtype=np.float64)).astype(np.float32)[None]
    if np.any(bo != 0):
        full += bo[None, None, :]
    return full
